# revision 18
# baseline (speedup 1.0000x reference)
"""Trainium2 Bass kernel for nn_Actor (MLP head + KL-constrained mixture Newton solve).

Contract: kernel(**inputs) takes FULL unsharded inputs (as produced by
setup_inputs()) and returns the FULL output tuple
(softmax_probs [8192,4096] f32, sq_kl_err [8192,1] f32), matching reference().

Strategy:
  - Pure data parallel over batch: 8 cores x 1024 rows each.
  - Per core, rows are processed in 128-row tiles, 4 groups of 2 tiles.
  - MLP runs on the TensorEngine in transposed layout (hid on partitions);
    the last layer emits normal layout [rows, actions] chunks into PSUM,
    with b2-bias and log_base_dist folded in via extra matmuls.
  - The 20-step Newton solve runs SBUF-resident per tile with state
    {amb = a-b, b, r = amb/b}:
        lp   = Ln(alpha*r + 1)                (ScalarE, fused scale)
        A    = sum(amb*lp), B = sum(b*lp)     (VectorE tensor_tensor_reduce)
        kl   = B + alpha*A ;  d = A + T0
        alpha update chain (tiny per-row ops on GpSimd/Pool engine)
  - Final: p = b + alpha*amb (with accum -> Px), probs = p/Px,
    kl_f = sum(p*lp_f), out2 = (kl_f - kl_target)^2.
"""

import math
import os
import sys

import numpy as np

# Prefer the b16 neuronxcc build: its walrus supports the multi-wait BIR
# instructions that concourse/tile emits (the stock walrus rejects them with
# "Too many sync wait commands").
_B16_SP = (
    "/nix/store/wxap7svlj45h0lfm31d1axjjnzyl6qsy-b16-bazel-unstable-cc-"
    "2026-05-04-9a3fa1f3-rt-2026-05-04-ade39e0a/lib/python3.13/site-packages"
)
if os.path.isdir(_B16_SP) and _B16_SP not in sys.path:
    sys.path.insert(0, _B16_SP)

sys.path.insert(0, "/opt/trn_rl_repo")

import concourse.bass as bass  # noqa: E402
import concourse.tile as tile  # noqa: E402
from concourse import bacc, mybir  # noqa: E402

F32 = mybir.dt.float32
ALU = mybir.AluOpType
ACTF = mybir.ActivationFunctionType
AX = mybir.AxisListType

N_CORES = 8
B_FULL = 8192
IN_DIM = 64
HID = 256
OUT = 4096
BC = B_FULL // N_CORES          # rows per core = 1024
P = 128                          # partition tile rows
N_TILES = BC // P                # 8
GROUP = 2                        # row-tiles per group
N_GROUPS = N_TILES // GROUP      # 4
N_STEPS = int(os.environ.get("KERNEL_N_STEPS", "20"))
DBG_STAGE = os.environ.get("KERNEL_DBG_STAGE", "full")  # setup|newton|full
NCH = OUT // 512                 # 8 action chunks of 512
CH = 512

_PROGRAM_CACHE: dict = {}


def _build_program(log_al: float) -> bass.Bass:
    nc = bacc.Bacc("TRN2", target_bir_lowering=False, debug=False)

    # ---- DRAM I/O ----
    xaug_d = nc.dram_tensor("xaug", [IN_DIM + 1, BC], F32, kind="ExternalInput")
    logb_d = nc.dram_tensor("logb", [BC, OUT], F32, kind="ExternalInput")
    klb_d = nc.dram_tensor("klb", [BC, 1], F32, kind="ExternalInput")
    w0t_d = nc.dram_tensor("w0t", [IN_DIM + 1, HID], F32, kind="ExternalInput")
    w1t_d = nc.dram_tensor("w1t", [HID, HID], F32, kind="ExternalInput")
    # w2aug = [W2.T ; b2] -> [257, 4097]
    w2aug_d = nc.dram_tensor("w2aug", [HID + 1, OUT + 1], F32, kind="ExternalInput")
    b0c_d = nc.dram_tensor("b0c", [P, HID // P], F32, kind="ExternalInput")
    b1c_d = nc.dram_tensor("b1c", [P, HID // P], F32, kind="ExternalInput")
    ident_d = nc.dram_tensor("ident", [P, P], F32, kind="ExternalInput")
    ones1_d = nc.dram_tensor("ones1", [1, P], F32, kind="ExternalInput")

    probs_d = nc.dram_tensor("probs", [BC, OUT], F32, kind="ExternalOutput")
    sqerr_d = nc.dram_tensor("sqerr", [BC, 1], F32, kind="ExternalOutput")

    with tile.TileContext(nc) as tc:
        from contextlib import ExitStack

        with ExitStack() as ctx:
            consts = ctx.enter_context(tc.tile_pool(name="consts", bufs=1))
            state = ctx.enter_context(tc.tile_pool(name="state", bufs=2))
            bigp = ctx.enter_context(tc.tile_pool(name="bigp", bufs=3))
            iop = ctx.enter_context(tc.tile_pool(name="iop", bufs=2))
            w2p = ctx.enter_context(tc.tile_pool(name="w2p", bufs=3))
            mlpp = ctx.enter_context(tc.tile_pool(name="mlpp", bufs=2))
            smallp = ctx.enter_context(tc.tile_pool(name="smallp", bufs=2))
            persm = ctx.enter_context(tc.tile_pool(name="persm", bufs=1))
            psmm = ctx.enter_context(
                tc.tile_pool(name="psmm", bufs=3, space="PSUM")
            )

            # ---- load constants ----
            xaug = consts.tile([IN_DIM + 1, BC], F32, name="xaug_sb")
            nc.sync.dma_start(xaug[:, :], xaug_d[:, :])
            w0t = consts.tile([IN_DIM + 1, HID], F32, name="w0t_sb")
            nc.sync.dma_start(w0t[:, :], w0t_d[:, :])
            w1t0 = consts.tile([P, HID], F32, name="w1t0_sb")
            nc.sync.dma_start(w1t0[:, :], w1t_d[0:P, :])
            w1t1 = consts.tile([P, HID], F32, name="w1t1_sb")
            nc.sync.dma_start(w1t1[:, :], w1t_d[P : 2 * P, :])
            b0c = consts.tile([P, HID // P], F32, name="b0c_sb")
            nc.sync.dma_start(b0c[:, :], b0c_d[:, :])
            b1c = consts.tile([P, HID // P], F32, name="b1c_sb")
            nc.sync.dma_start(b1c[:, :], b1c_d[:, :])
            ident = consts.tile([P, P], F32, name="ident_sb")
            nc.sync.dma_start(ident[:, :], ident_d[:, :])
            ones1 = consts.tile([1, P], F32, name="ones1_sb")
            nc.sync.dma_start(ones1[:, :], ones1_d[:, :])
            # W2 head column pieces: [hid-chunk0, hid-chunk1, bias] as 3 cols
            w2h = consts.tile([P, 3], F32, name="w2h_sb")
            nc.sync.dma_start(w2h[:, 0:1], w2aug_d[0:P, 0:1])
            nc.sync.dma_start(w2h[:, 1:2], w2aug_d[P : 2 * P, 0:1])
            nc.sync.dma_start(w2h[0:1, 2:3], w2aug_d[2 * P : 2 * P + 1, 0:1])
            # per-partition constant: -log(actions_left), used as sigmoid bias
            negshift = consts.tile([P, 1], F32, name="negshift_sb")
            nc.gpsimd.memset(negshift[:, :], -log_al)

            for g in range(N_GROUPS):
                gsl = slice(g * GROUP * P, (g + 1) * GROUP * P)

                # ---- MLP layers 0/1 (transposed layout, group of 256 rows) ----
                xg = xaug[:, gsl]
                t1 = []
                for m in range(2):
                    p1 = psmm.tile([P, GROUP * P], F32, name=f"p1_{g}_{m}", tag="psmm")
                    nc.tensor.matmul(
                        p1[:, :], w0t[:, m * P : (m + 1) * P], xg, start=True, stop=True
                    )
                    t1m = mlpp.tile([P, GROUP * P], F32, name=f"t1_{g}_{m}", tag=f"t1_{m}")
                    nc.scalar.activation(
                        t1m[:, :], p1[:, :], ACTF.Tanh, bias=b0c[:, m : m + 1]
                    )
                    t1.append(t1m)
                t2 = []
                for mo in range(2):
                    p2 = psmm.tile([P, GROUP * P], F32, name=f"p2_{g}_{mo}", tag="psmm")
                    for mi in range(2):
                        w1sb = w1t0 if mi == 0 else w1t1
                        nc.tensor.matmul(
                            p2[:, :],
                            w1sb[:, mo * P : (mo + 1) * P],
                            t1[mi][:, :],
                            start=(mi == 0),
                            stop=(mi == 1),
                        )
                    t2m = mlpp.tile([P, GROUP * P], F32, name=f"t2_{g}_{mo}", tag=f"t2_{mo}")
                    nc.scalar.activation(
                        t2m[:, :], p2[:, :], ACTF.Tanh, bias=b1c[:, mo : mo + 1]
                    )
                    t2.append(t2m)

                # per-tile state handles for this group
                g_amb, g_b, g_r = [], [], []
                g_alpha, g_bot, g_top, g_k, g_T0 = [], [], [], [], []

                for tl in range(GROUP):
                    t = g * GROUP + tl
                    rsl = slice(t * P, (t + 1) * P)
                    lh = [t2[mi][:, tl * P : (tl + 1) * P] for mi in range(2)]

                    # logb tile
                    logb_t = iop.tile([P, OUT], F32, name=f"logb_{t}", tag="logb")
                    nc.sync.dma_start(logb_t[:, :], logb_d[rsl, :])

                    # ---- kl_target head ----
                    ph = psmm.tile([P, 1], F32, name=f"ph_{t}", tag="psmm")
                    nc.tensor.matmul(ph[:, :], lh[0], w2h[:, 0:1], start=True, stop=False)
                    nc.tensor.matmul(ph[:, :], lh[1], w2h[:, 1:2], start=False, stop=False)
                    nc.tensor.matmul(
                        ph[:, :], ones1[:, :], w2h[0:1, 2:3], start=False, stop=True
                    )
                    sig = smallp.tile([P, 1], F32, name=f"sig_{t}", tag="sig")
                    nc.scalar.activation(
                        sig[:, :], ph[:, :], ACTF.Sigmoid, bias=negshift[:, :]
                    )
                    klb_t = smallp.tile([P, 1], F32, name=f"klb_{t}", tag="klb")
                    nc.sync.dma_start(klb_t[:, :], klb_d[rsl, :])
                    k_t = persm.tile([P, 1], F32, name=f"k_{t}", tag=f"k_{t % (2*GROUP)}")
                    nc.vector.tensor_scalar(
                        k_t[:, :], sig[:, :], klb_t[:, :], 1e-6, ALU.mult, ALU.add
                    )

                    # ---- logits chunks -> loga_u in PSUM -> SBUF (amb slot) ----
                    amb = state.tile([P, OUT], F32, name=f"amb_{t}", tag="amb")
                    mparts = smallp.tile([P, NCH], F32, name=f"mparts_{t}", tag="mparts")
                    for a in range(NCH):
                        pc = psmm.tile([P, CH], F32, name=f"pc_{t}_{a}", tag="psmm")
                        for mi in range(2):
                            w2c = w2p.tile([P, CH], F32, name=f"w2c_{t}_{a}_{mi}", tag="w2k")
                            nc.sync.dma_start(
                                w2c[:, :],
                                w2aug_d[mi * P : (mi + 1) * P, 1 + a * CH : 1 + (a + 1) * CH],
                            )
                            nc.tensor.matmul(
                                pc[:, :], lh[mi], w2c[:, :], start=(mi == 0), stop=False
                            )
                        w2b = w2p.tile([1, CH], F32, name=f"w2b_{t}_{a}", tag="w2b")
                        nc.sync.dma_start(
                            w2b[:, :],
                            w2aug_d[2 * P : 2 * P + 1, 1 + a * CH : 1 + (a + 1) * CH],
                        )
                        nc.tensor.matmul(pc[:, :], ones1[:, :], w2b[:, :], start=False, stop=False)
                        nc.tensor.matmul(
                            pc[:, :],
                            ident[:, :],
                            logb_t[:, a * CH : (a + 1) * CH],
                            start=False,
                            stop=True,
                        )
                        # loga_u chunk -> SBUF (into amb slot for now)
                        nc.scalar.copy(amb[:, a * CH : (a + 1) * CH], pc[:, :])
                        # row max of chunk
                        nc.vector.tensor_reduce(
                            mparts[:, a : a + 1],
                            amb[:, a * CH : (a + 1) * CH],
                            AX.X,
                            ALU.max,
                        )
                    m_t = smallp.tile([P, 1], F32, name=f"m_{t}", tag="m")
                    nc.vector.tensor_reduce(m_t[:, :], mparts[:, :], AX.X, ALU.max)
                    negm = smallp.tile([P, 1], F32, name=f"negm_{t}", tag="negm")
                    nc.scalar.activation(negm[:, :], m_t[:, :], ACTF.Identity, scale=-1.0)

                    # ea = exp(loga_u - m) in place, accumulate S
                    S_t = smallp.tile([P, 1], F32, name=f"S_{t}", tag="S")
                    nc.scalar.activation(
                        amb[:, :], amb[:, :], ACTF.Exp, bias=negm[:, :], accum_out=S_t[:, :]
                    )
                    Sinv = smallp.tile([P, 1], F32, name=f"Sinv_{t}", tag="Sinv")
                    nc.vector.reciprocal(Sinv[:, :], S_t[:, :])

                    # b = exp(logb)
                    b_t = state.tile([P, OUT], F32, name=f"b_{t}", tag="b")
                    nc.scalar.activation(b_t[:, :], logb_t[:, :], ACTF.Exp)
                    # binv = exp(-logb), in place over logb slot
                    nc.scalar.activation(logb_t[:, :], logb_t[:, :], ACTF.Exp, scale=-1.0)

                    # amb = ea*Sinv - b  (in place), accumulate T0
                    T0 = smallp.tile([P, 1], F32, name=f"T0_{t}", tag="T0")
                    nc.vector.scalar_tensor_tensor(
                        amb[:, :],
                        amb[:, :],
                        Sinv[:, :],
                        b_t[:, :],
                        ALU.mult,
                        ALU.subtract,
                        accum_out=T0[:, :],
                    )
                    # T0e = T0 + 1e-12 (newton denominator bias, precomputed)
                    T0e = persm.tile([P, 1], F32, name=f"T0e_{t}", tag=f"T0e_{t % (2*GROUP)}")
                    nc.vector.tensor_scalar(T0e[:, :], T0[:, :], 1e-12, None, ALU.add)
                    # r = amb * binv
                    r_t = state.tile([P, OUT], F32, name=f"r_{t}", tag="r")
                    nc.vector.tensor_tensor(r_t[:, :], amb[:, :], logb_t[:, :], ALU.mult)

                    # newton scalar state
                    alpha = persm.tile([P, 1], F32, name=f"alpha_{t}", tag=f"alpha_{t % (2*GROUP)}")
                    nc.gpsimd.memset(alpha[:, :], 1.0)
                    bot = persm.tile([P, 1], F32, name=f"bot_{t}", tag=f"bot_{t % (2*GROUP)}")
                    nc.gpsimd.memset(bot[:, :], 0.0)
                    top = persm.tile([P, 1], F32, name=f"top_{t}", tag=f"top_{t % (2*GROUP)}")
                    nc.gpsimd.memset(top[:, :], 1.0)

                    g_amb.append(amb)
                    g_b.append(b_t)
                    g_r.append(r_t)
                    g_alpha.append(alpha)
                    g_bot.append(bot)
                    g_top.append(top)
                    g_k.append(k_t)
                    g_T0.append(T0e)

                if DBG_STAGE == "setup":
                    for tl in range(GROUP):
                        t = g * GROUP + tl
                        rsl = slice(t * P, (t + 1) * P)
                        nc.sync.dma_start(probs_d[rsl, :], g_r[tl][:, :])
                        nc.sync.dma_start(sqerr_d[rsl, :], g_k[tl][:, :])
                    continue

                # ---- Newton iterations (pipelined across the 2 tiles) ----
                for step in range(N_STEPS):
                    for tl in range(GROUP):
                        t = g * GROUP + tl
                        amb, b_t, r_t = g_amb[tl], g_b[tl], g_r[tl]
                        alpha, bot, top = g_alpha[tl], g_bot[tl], g_top[tl]
                        k_t, T0 = g_k[tl], g_T0[tl]

                        lp = bigp.tile([P, OUT], F32, name=f"lp_{t}_{step}", tag="lp")
                        nc.scalar.activation(
                            lp[:, :], r_t[:, :], ACTF.Ln, bias=1.0, scale=alpha[:, :]
                        )

                        # product+reduce in one DVE pass: out is a broadcast
                        # dummy (stride-0 writes), accum_out carries the sum.
                        dA = smallp.tile([P, 1], F32, name=f"dA_{t}_{step}", tag="dA")
                        A = smallp.tile([P, 1], F32, name=f"A_{t}_{step}", tag="A")
                        nc.vector.scalar_tensor_tensor(
                            dA.broadcast_to((P, OUT)), amb[:, :], 1.0, lp[:, :],
                            ALU.mult, ALU.mult, accum_out=A[:, :],
                        )
                        dB = smallp.tile([P, 1], F32, name=f"dB_{t}_{step}", tag="dB")
                        Bb = smallp.tile([P, 1], F32, name=f"B_{t}_{step}", tag="B")
                        nc.vector.scalar_tensor_tensor(
                            dB.broadcast_to((P, OUT)), b_t[:, :], 1.0, lp[:, :],
                            ALU.mult, ALU.mult, accum_out=Bb[:, :],
                        )

                        # tiny per-row update chain: linear ops on ScalarE
                        # (Identity(x*scale + bias) with per-partition APs),
                        # comparisons/clip/reciprocal on VectorE.
                        def stile(nm):
                            return smallp.tile([P, 1], F32, name=f"{nm}_{t}_{step}", tag=nm)

                        kl = stile("kl")
                        nc.scalar.activation(
                            kl[:, :], A[:, :], ACTF.Identity,
                            scale=alpha[:, :], bias=Bb[:, :],
                        )
                        num = stile("num")
                        nc.scalar.activation(
                            num[:, :], kl[:, :], ACTF.Identity,
                            scale=-1.0, bias=k_t[:, :],
                        )
                        den = stile("den")
                        nc.scalar.activation(
                            den[:, :], A[:, :], ACTF.Identity, bias=T0[:, :]
                        )
                        rden = stile("rden")
                        nc.vector.reciprocal(rden[:, :], den[:, :])
                        araw = stile("araw")
                        nc.scalar.activation(
                            araw[:, :], num[:, :], ACTF.Identity,
                            scale=rden[:, :], bias=alpha[:, :],
                        )
                        ge = stile("ge")
                        nc.vector.tensor_tensor(ge[:, :], k_t[:, :], kl[:, :], ALU.is_ge)
                        le = stile("le")
                        nc.vector.tensor_tensor(le[:, :], k_t[:, :], kl[:, :], ALU.is_le)
                        tb = stile("tb")
                        nc.scalar.activation(
                            tb[:, :], bot[:, :], ACTF.Identity,
                            scale=-1.0, bias=alpha[:, :],
                        )
                        nc.scalar.activation(
                            bot[:, :], tb[:, :], ACTF.Identity,
                            scale=ge[:, :], bias=bot[:, :],
                        )
                        tt = stile("tt")
                        nc.scalar.activation(
                            tt[:, :], top[:, :], ACTF.Identity,
                            scale=-1.0, bias=alpha[:, :],
                        )
                        nc.scalar.activation(
                            top[:, :], tt[:, :], ACTF.Identity,
                            scale=le[:, :], bias=top[:, :],
                        )
                        t16 = stile("t16")
                        nc.scalar.activation(
                            t16[:, :], top[:, :], ACTF.Identity, scale=1.0 / 16.0
                        )
                        lo = stile("lo")
                        nc.scalar.activation(
                            lo[:, :], bot[:, :], ACTF.Identity,
                            scale=15.0 / 16.0, bias=t16[:, :],
                        )
                        t05 = stile("t05")
                        nc.scalar.activation(
                            t05[:, :], top[:, :], ACTF.Identity, scale=0.5
                        )
                        hi = stile("hi")
                        nc.scalar.activation(
                            hi[:, :], bot[:, :], ACTF.Identity,
                            scale=0.5, bias=t05[:, :],
                        )
                        ac = stile("ac")
                        nc.vector.tensor_tensor(ac[:, :], araw[:, :], lo[:, :], ALU.max)
                        nc.vector.tensor_tensor(alpha[:, :], ac[:, :], hi[:, :], ALU.min)

                # ---- finalize tiles of this group ----
                for tl in range(GROUP):
                    t = g * GROUP + tl
                    rsl = slice(t * P, (t + 1) * P)
                    amb, b_t, r_t = g_amb[tl], g_b[tl], g_r[tl]
                    alpha, k_t = g_alpha[tl], g_k[tl]

                    lpf = bigp.tile([P, OUT], F32, name=f"lpf_{t}", tag="lp")
                    nc.scalar.activation(
                        lpf[:, :], r_t[:, :], ACTF.Ln, bias=1.0, scale=alpha[:, :]
                    )
                    p_t = bigp.tile([P, OUT], F32, name=f"p_{t}", tag="lp")
                    Px = smallp.tile([P, 1], F32, name=f"Px_{t}", tag="Px")
                    nc.vector.scalar_tensor_tensor(
                        p_t[:, :], amb[:, :], alpha[:, :], b_t[:, :],
                        ALU.mult, ALU.add, accum_out=Px[:, :],
                    )
                    dK = smallp.tile([P, 1], F32, name=f"dK_{t}", tag="dK")
                    klf = smallp.tile([P, 1], F32, name=f"klf_{t}", tag="klf")
                    nc.vector.scalar_tensor_tensor(
                        dK.broadcast_to((P, OUT)), p_t[:, :], 1.0, lpf[:, :],
                        ALU.mult, ALU.mult, accum_out=klf[:, :],
                    )
                    Pxi = smallp.tile([P, 1], F32, name=f"Pxi_{t}", tag="Pxi")
                    nc.vector.reciprocal(Pxi[:, :], Px[:, :])
                    nc.vector.tensor_scalar(
                        p_t[:, :], p_t[:, :], Pxi[:, :], None, ALU.mult
                    )
                    nc.sync.dma_start(probs_d[rsl, :], p_t[:, :])

                    ef = smallp.tile([P, 1], F32, name=f"ef_{t}", tag="ef")
                    nc.vector.tensor_tensor(ef[:, :], klf[:, :], k_t[:, :], ALU.subtract)
                    e2 = smallp.tile([P, 1], F32, name=f"e2_{t}", tag="e2")
                    nc.vector.tensor_tensor(e2[:, :], ef[:, :], ef[:, :], ALU.mult)
                    nc.sync.dma_start(sqerr_d[rsl, :], e2[:, :])

    nc.compile()
    return nc


def _get_program(log_al: float) -> bass.Bass:
    key = round(log_al, 12)
    if key not in _PROGRAM_CACHE:
        _PROGRAM_CACHE[key] = _build_program(log_al)
    return _PROGRAM_CACHE[key]


def _host_prep(inputs):
    x = np.ascontiguousarray(np.asarray(inputs["x"], dtype=np.float32))
    klb = np.ascontiguousarray(np.asarray(inputs["kl_budget"], dtype=np.float32))
    logb = np.asarray(inputs["log_base_dist"], dtype=np.float32)
    W0 = np.asarray(inputs["W0"], dtype=np.float32)
    b0 = np.asarray(inputs["b0"], dtype=np.float32)
    W1 = np.asarray(inputs["W1"], dtype=np.float32)
    b1 = np.asarray(inputs["b1"], dtype=np.float32)
    W2 = np.asarray(inputs["W2"], dtype=np.float32)
    b2 = np.asarray(inputs["b2"], dtype=np.float32)
    al = float(np.asarray(inputs["actions_left"]))
    log_al = float(math.log(al)) if al > 0 else -745.0

    w0t = np.ascontiguousarray(W0.T)                    # [65, 256]
    w1t = np.ascontiguousarray(W1.T)                    # [256, 256]
    w2aug = np.ascontiguousarray(
        np.concatenate([W2.T, b2[None, :]], axis=0)     # [257, 4097]
    )
    b0c = np.ascontiguousarray(b0.reshape(HID // P, P).T)  # [128, 2]
    b1c = np.ascontiguousarray(b1.reshape(HID // P, P).T)
    ident = np.eye(P, dtype=np.float32)
    ones1 = np.ones((1, P), dtype=np.float32)

    shared = {
        "w0t": w0t, "w1t": w1t, "w2aug": w2aug,
        "b0c": b0c, "b1c": b1c, "ident": ident, "ones1": ones1,
    }
    in_maps = []
    for c in range(N_CORES):
        sl = slice(c * BC, (c + 1) * BC)
        xaug = np.ascontiguousarray(
            np.concatenate([x[sl].T, klb[sl].T], axis=0)  # [65, 1024]
        )
        m = {
            "xaug": xaug,
            "logb": np.ascontiguousarray(logb[sl]),
            "klb": klb[sl],
        }
        m.update(shared)
        in_maps.append(m)
    return log_al, in_maps


def kernel(**inputs):
    from concourse.bass_utils import run_bass_kernel_spmd

    log_al, in_maps = _host_prep(inputs)
    nc = _get_program(log_al)
    res = run_bass_kernel_spmd(nc, in_maps, core_ids=list(range(N_CORES)))
    probs = np.concatenate([res.results[c]["probs"] for c in range(N_CORES)], axis=0)
    sqerr = np.concatenate([res.results[c]["sqerr"] for c in range(N_CORES)], axis=0)
    return probs, sqerr
